# revision 11
# baseline (speedup 1.0000x reference)
"""CrossAttentionFusion kernel for Trainium2 (8 NeuronCores, data-parallel over batch).

Reference computation (per batch element, S=2048, D=512, HID=256):
  Q = l @ Wq + bq ; K = a @ Wk + bk ; V = a @ Wv + bv
  P = softmax(Q K^T / sqrt(D)) ; O = P @ V
  fused_l = gl*O + (2-gl)*l          (gl = sigmoid(alpha_l))
  fused_a = (1+ga)*a                 (ga = sigmoid(alpha_a))
  w = sigmoid(relu(v @ W1 + b1) @ W2 + b2) ; fused_v = w*v
  out = concat([fused_l, fused_a, fused_v], -1)     # [S, 3D]

Kernel strategy (per core, one batch element):
  - everything matmul-shaped (projections, MLP, scores, PV) runs in fp8 e4m3
    with DoubleRow matmuls (2 fp8 weights per PE cell -> 2x MACs/cycle);
    weights arrive pre-cast from the host. Elementwise epilogues read the
    original fp32 activations so residual paths stay exact.
  - softmax skips the max pass (scores bounded): P = exp(scale*s - 1.5); the
    offset cancels in the normalization and keeps P well under the TRN fp8
    max of 240. Row sums come from an M=1 DoubleRow matmul against a ones
    vector, transposed back to [128,1] via tiny K=1 matmuls.
  - x^T layouts are produced on TensorE by normal-mode matmuls against an fp8
    identity. l-transpose PSUM->SBUF copies ride the scalar engine, v-copies
    the vector engine, placed in each queue so no matmul waits on a copy
    issued in the same slot.
  - software pipeline: [l0 | a-stream + qb0 scores | v0 | qb0] then slots of
    [lT/Q-proj i, vT/MLP i, scores i, rowsum i, {loads+casts i+1}, PV i].
    DMA+cast for slot i+1 is emitted before slot i's PV/epilogue so the
    casts clear the ACT/DVE queues before the epilogue floods them.
  - a and l stay resident in SBUF in fp32 (a_keep/l_keep): no input is read
    twice. Input tiles alternate between the two HWDGE queues; outputs are
    spread over gpsimd/scalar/sync queues to shorten the final drain.
"""

import math
from contextlib import ExitStack

import ml_dtypes
import numpy as np

import concourse.bass as bass
import concourse.tile as tile
from concourse import bacc, mybir
from concourse.bass_utils import run_bass_kernel_spmd

B, S, D = 8, 2048, 512
HID = D // 2
P = 128  # partitions
NS = S // P          # 16 s-tiles
NC = D // P          # 4 d-chunks
NH = HID // P        # 2 hid-chunks
QB = 512             # q-block / s-chunk size
NQB = S // QB        # 4 chunks
TPC = QB // P        # 4 s-tiles per chunk
SCALE = 1.0 / math.sqrt(D)
OFF = 1.5            # exp offset, cancels in softmax normalization

F32 = mybir.dt.float32
BF16 = mybir.dt.bfloat16
FP8 = mybir.dt.float8e4
DR = mybir.MatmulPerfMode.DoubleRow


def build_kernel(gl: float, ga: float, b2val: float):
    nc = bacc.Bacc("TRN2", target_bir_lowering=False, debug=False, num_devices=8)

    a_t = nc.dram_tensor("a_t", [NS, P, D], F32, kind="ExternalInput").ap()
    l_t = nc.dram_tensor("l_t", [NS, P, D], F32, kind="ExternalInput").ap()
    v_t = nc.dram_tensor("v_t", [NS, P, D], F32, kind="ExternalInput").ap()
    wq = nc.dram_tensor("wq", [NC, P, D], FP8, kind="ExternalInput").ap()
    wk = nc.dram_tensor("wk", [NC, P, D], FP8, kind="ExternalInput").ap()
    wv = nc.dram_tensor("wv", [NC, P, D], FP8, kind="ExternalInput").ap()
    w1 = nc.dram_tensor("w1", [NC, P, HID], FP8, kind="ExternalInput").ap()
    w2 = nc.dram_tensor("w2", [P, 2, 16], FP8, kind="ExternalInput").ap()
    bq = nc.dram_tensor("bq", [P, NC], F32, kind="ExternalInput").ap()
    bk = nc.dram_tensor("bk", [P, NC], F32, kind="ExternalInput").ap()
    bv = nc.dram_tensor("bv", [1, D], F32, kind="ExternalInput").ap()
    b1 = nc.dram_tensor("b1", [P, NH], F32, kind="ExternalInput").ap()
    ident8_in = nc.dram_tensor("ident8_in", [P, P], FP8, kind="ExternalInput").ap()
    out = nc.dram_tensor("out", [NS, P, 3 * D], F32, kind="ExternalOutput").ap()

    with tile.TileContext(nc) as tc:
        _emit(tc, a_t, l_t, v_t, wq, wk, wv, w1, w2, bq, bk, bv, b1,
              ident8_in, out, gl, ga, b2val)

    nc.compile()
    return nc


def _emit(tc, a_t, l_t, v_t, wq, wk, wv, w1, w2, bq, bk, bv, b1,
          ident8_in, out, gl, ga, b2val):
    nc = tc.nc
    AF = mybir.ActivationFunctionType
    OP = mybir.AluOpType

    ctx = ExitStack()
    consts = ctx.enter_context(tc.tile_pool(name="consts", bufs=1))
    persist = ctx.enter_context(tc.tile_pool(name="persist", bufs=1))
    stage = ctx.enter_context(tc.tile_pool(name="stage", bufs=2))

    # ---- constants ----
    ident8 = consts.tile([P, P], FP8, tag="ident8")
    nc.sync.dma_start(out=ident8[:], in_=ident8_in)

    # HAM warm-up: dependency-free matmuls to open the PE clock gate while the
    # first input tiles stream in.
    warm_in = consts.tile([P, P], BF16, tag="warm_in")
    nc.vector.memset(warm_in[:], 0.5)
    with tc.tile_pool(name="psum_warm", bufs=1, space="PSUM") as psum_warm:
        wps = psum_warm.tile([P, P], F32, tag="warm")
        for _ in range(48):
            nc.tensor.matmul(
                wps[:], lhsT=warm_in[:], rhs=warm_in[:], start=True, stop=True
            )

    psum_mm = ctx.enter_context(tc.tile_pool(name="psum_mm", bufs=4, space="PSUM"))
    psum_att = ctx.enter_context(tc.tile_pool(name="psum_att", bufs=2, space="PSUM"))
    psum_r = ctx.enter_context(tc.tile_pool(name="psum_r", bufs=1, space="PSUM"))

    # biases / small consts
    bq_sb = consts.tile([P, NC], F32, tag="bq_sb")
    bk_sb = consts.tile([P, NC], F32, tag="bk_sb")
    b1_sb = consts.tile([P, NH], F32, tag="b1_sb")
    nc.gpsimd.dma_start(out=bq_sb[:], in_=bq)
    nc.gpsimd.dma_start(out=bk_sb[:], in_=bk)
    nc.gpsimd.dma_start(out=b1_sb[:], in_=b1)
    bv_bc = consts.tile([P, D], F32, tag="bv_bc")
    bv_bcast_ap = bass.AP(tensor=bv.tensor, offset=bv.offset, ap=[[0, P], bv.ap[1]])
    nc.gpsimd.dma_start(out=bv_bc[:], in_=bv_bcast_ap)

    ones8 = consts.tile([P, 2, 16], FP8, tag="ones8")
    nc.vector.memset(ones8[:], 1.0)
    onef = consts.tile([1, 1], F32, tag="onef")
    nc.vector.memset(onef[:], 1.0)
    exp_bias = consts.tile([P, 1], F32, tag="exp_bias")
    nc.vector.memset(exp_bias[:], -OFF)
    b2h = consts.tile([P, 1], F32, tag="b2h")
    nc.vector.memset(b2h[:], 0.5 * b2val)

    # weights (pre-cast host-side): wq first, it opens the pipeline.
    wq8 = consts.tile([P, NC, D], FP8, tag="wq8")
    wk8 = consts.tile([P, NC, D], FP8, tag="wk8")
    wv8 = consts.tile([P, NC, D], FP8, tag="wv8")
    w18 = consts.tile([P, NC, HID], FP8, tag="w18")
    w28 = consts.tile([P, 2, 16], FP8, tag="w28")
    for dram, sb in ((wq, wq8), (wk, wk8), (wv, wv8)):
        for c in range(NC):
            nc.gpsimd.dma_start(out=sb[:, c, :], in_=dram[c])
    for c in range(NC):
        nc.gpsimd.dma_start(out=w18[:, c, :], in_=w1[c])
    nc.gpsimd.dma_start(out=w28[:], in_=w2)

    # ---- persistent activations ----
    kT = persist.tile([P, NC, S], FP8, tag="kT")          # K^T [d, s] fp8
    qT = persist.tile([P, NC, S], FP8, tag="qT")          # Q^T [d, s] fp8
    v_sb = persist.tile([P, NS, D], FP8, tag="v_sb")      # V natural fp8
    a_keep = persist.tile([P, NS, D], F32, tag="a_keep")  # a resident fp32
    l_keep = persist.tile([P, NS, D], F32, tag="l_keep")  # l resident fp32
    w_sb = persist.tile([P, NS], F32, tag="w_sb")         # visual weights

    def in_dma(dst, src, st4):
        """Input tile load, alternating between the two HWDGE queues."""
        if st4 % 2 == 0:
            nc.sync.dma_start(out=dst, in_=src)
        else:
            nc.scalar.dma_start(out=dst, in_=src)

    def transpose_tile(x8, dstT, st4, on_act):
        """dstT[:, :, st4*P:(st4+1)*P] = transpose of [P, D] fp8 tile x8 via
        PE identity matmuls."""
        ps = psum_mm.tile([P, NC * P], F32, tag="mm", name=f"tp{st4}")
        for c in range(NC):
            nc.tensor.matmul(
                ps[:, c * P : (c + 1) * P],
                lhsT=x8[:, c * P : (c + 1) * P],
                rhs=ident8[:],
                start=True,
                stop=True,
            )
        dst = dstT[:, :, st4 * P : (st4 + 1) * P]
        if on_act:
            nc.scalar.copy(dst, ps[:])
        else:
            nc.vector.tensor_copy(dst, ps[:])

    def proj_write(dst, ps, bias_sb, co):
        if co % 2 == 0:
            nc.scalar.activation(
                out=dst, in_=ps[:], func=AF.Identity,
                bias=bias_sb[:, co : co + 1], scale=1.0,
            )
        else:
            nc.vector.tensor_scalar_add(
                out=dst, in0=ps[:], scalar1=bias_sb[:, co : co + 1]
            )

    def cast8(dst, src, st4):
        if st4 % 2 == 0:
            nc.scalar.copy(dst, src)
        else:
            nc.vector.tensor_copy(dst, src)

    with (
        tc.tile_pool(name="chunkA", bufs=2) as apool,
        tc.tile_pool(name="chunkL", bufs=2) as lpool,
        tc.tile_pool(name="chunkV", bufs=2) as vpool,
        tc.tile_pool(name="ppool", bufs=2) as ppool,
    ):
        def l_load_cast(lc):
            l8s = []
            for st4 in range(TPC):
                st = lc * TPC + st4
                in_dma(l_keep[:, st, :], l_t[st], st4)
                l8 = stage.tile([P, D], FP8, tag="l8", bufs=8, name=f"l8_{st}")
                cast8(l8[:], l_keep[:, st, :], st4)
                l8s.append(l8)
            return l8s

        def l_transp_proj(lc, l8s):
            """Transpose l chunk (copies on ACT) and project to qT."""
            lT = lpool.tile([P, NC, QB], FP8, tag="lT", name=f"lT{lc}")
            for st4 in range(TPC):
                transpose_tile(l8s[st4], lT, st4, on_act=True)
            for co in range(NC):
                ps = psum_mm.tile([P, QB], F32, tag="mm", name=f"q{lc}{co}")
                for cp in (0, 2):
                    nc.tensor.matmul(
                        ps[:],
                        lhsT=wq8[:, cp : cp + 2, co * P : (co + 1) * P],
                        rhs=lT[:, cp : cp + 2, :],
                        start=(cp == 0),
                        stop=(cp == 2),
                        perf_mode=DR,
                    )
                proj_write(qT[:, co, lc * QB : (lc + 1) * QB], ps, bq_sb, co)

        def v_load_cast(vc):
            vfs, v8s = [], []
            for st4 in range(TPC):
                st = vc * TPC + st4
                vf = stage.tile([P, D], F32, tag="vf", bufs=10, name=f"vf{st}")
                in_dma(vf[:], v_t[st], st4)
                vfs.append(vf)
                v8 = stage.tile([P, D], FP8, tag="v8", bufs=8, name=f"v8_{st}")
                cast8(v8[:], vf[:], st4 + 1)
                v8s.append(v8)
            return vfs, v8s

        def v_rest(vc, vfs, v8s):
            """Transpose v (copies on DVE), fp8 MLP, fused_v writes."""
            vT = vpool.tile([P, NC, QB], FP8, tag="vT", name=f"vT{vc}")
            hT = vpool.tile([P, NH, QB], FP8, tag="hT", name=f"hT{vc}")
            for st4 in range(TPC):
                transpose_tile(v8s[st4], vT, st4, on_act=False)
            for ch in range(NH):
                ps = psum_mm.tile([P, QB], F32, tag="mm", name=f"h{vc}{ch}")
                for cp in (0, 2):
                    nc.tensor.matmul(
                        ps[:],
                        lhsT=w18[:, cp : cp + 2, ch * P : (ch + 1) * P],
                        rhs=vT[:, cp : cp + 2, :],
                        start=(cp == 0),
                        stop=(cp == 2),
                        perf_mode=DR,
                    )
                nc.scalar.activation(
                    out=hT[:, ch, :],
                    in_=ps[:],
                    func=AF.Relu,
                    bias=b1_sb[:, ch : ch + 1],
                    scale=1.0,
                )
            for st4 in range(TPC):
                st = vc * TPC + st4
                psw = psum_mm.tile([P, QB], F32, tag="mm", name=f"w{vc}{st4}")
                nc.tensor.matmul(
                    psw[:, 0:1],
                    lhsT=hT[:, 0:NH, st4 * P : (st4 + 1) * P],
                    rhs=w28[:, :, 0:1],
                    start=True,
                    stop=True,
                    perf_mode=DR,
                )
                wt = stage.tile([P, 1], F32, tag="wt", bufs=2)
                nc.scalar.activation(
                    out=wt[:], in_=psw[:, 0:1], func=AF.Tanh, bias=b2h[:], scale=0.5
                )
                nc.vector.tensor_scalar(
                    out=w_sb[:, st : st + 1],
                    in0=wt[:],
                    scalar1=0.5,
                    scalar2=0.5,
                    op0=OP.mult,
                    op1=OP.add,
                )
                ov = stage.tile([P, D], F32, tag="ov", bufs=3)
                nc.vector.tensor_scalar_mul(
                    out=ov[:], in0=vfs[st4][:], scalar1=w_sb[:, st : st + 1]
                )
                nc.sync.dma_start(out=out[st, :, 2 * D : 3 * D], in_=ov[:])

        def a_chunk(sc):
            """Load a chunk into a_keep, transpose, K & V projections."""
            aT = apool.tile([P, NC, QB], FP8, tag="aT", name=f"aT{sc}")
            for st4 in range(TPC):
                st = sc * TPC + st4
                in_dma(a_keep[:, st, :], a_t[st], st4)
                a8 = stage.tile([P, D], FP8, tag="a8", bufs=4)
                cast8(a8[:], a_keep[:, st, :], st4)
                transpose_tile(a8, aT, st4, on_act=(st4 % 2 == 0))
            for co in range(NC):
                ps = psum_mm.tile([P, QB], F32, tag="mm", name=f"k{sc}{co}")
                for cp in (0, 2):
                    nc.tensor.matmul(
                        ps[:],
                        lhsT=wk8[:, cp : cp + 2, co * P : (co + 1) * P],
                        rhs=aT[:, cp : cp + 2, :],
                        start=(cp == 0),
                        stop=(cp == 2),
                        perf_mode=DR,
                    )
                proj_write(kT[:, co, sc * QB : (sc + 1) * QB], ps, bk_sb, co)
            for st4 in range(TPC):
                st = sc * TPC + st4
                ps = psum_mm.tile([P, QB], F32, tag="mm", name=f"v{sc}{st4}")
                for cp in (0, 2):
                    nc.tensor.matmul(
                        ps[:],
                        lhsT=aT[:, cp : cp + 2, st4 * P : (st4 + 1) * P],
                        rhs=wv8[:, cp : cp + 2, :],
                        start=(cp == 0),
                        stop=(cp == 2),
                        perf_mode=DR,
                    )
                nc.vector.tensor_add(v_sb[:, st, 0:D], ps[:], bv_bc[:])

        def scores_group(qb, pT, kts):
            for kt in kts:
                ps = psum_mm.tile([P, QB], F32, tag="mm", name=f"s{qb}{kt}")
                for cp in (0, 2):
                    nc.tensor.matmul(
                        ps[:],
                        lhsT=kT[:, cp : cp + 2, kt * P : (kt + 1) * P],
                        rhs=qT[:, cp : cp + 2, qb * QB : (qb + 1) * QB],
                        start=(cp == 0),
                        stop=(cp == 2),
                        perf_mode=DR,
                    )
                nc.scalar.activation(
                    out=pT[:, kt, :], in_=ps[:], func=AF.Exp,
                    bias=exp_bias[:], scale=SCALE,
                )

        def rowsum(qb, pT):
            """Row sums via M=1 DoubleRow + tiny K=1 transposes -> rinv tiles."""
            psr = psum_r.tile([1, QB], F32, tag="r", name=f"r{qb}")
            for kp in range(NS // 2):
                nc.tensor.matmul(
                    psr[:],
                    lhsT=ones8[:, :, 0:1],
                    rhs=pT[:, 2 * kp : 2 * kp + 2, :],
                    start=(kp == 0),
                    stop=(kp == NS // 2 - 1),
                    perf_mode=DR,
                )
            rsb = stage.tile([1, QB], F32, tag="rsb", bufs=2)
            nc.scalar.copy(rsb[:], psr[:])
            rinvs = []
            for qt in range(TPC):
                pst = psum_mm.tile([P, QB], F32, tag="mm", name=f"rt{qb}{qt}")
                nc.tensor.matmul(
                    pst[:, 0:1],
                    lhsT=rsb[:, qt * P : (qt + 1) * P],
                    rhs=onef[:],
                    start=True,
                    stop=True,
                )
                rinv = stage.tile([P, 1], F32, tag="rinv", bufs=8, name=f"ri{qb}{qt}")
                nc.vector.reciprocal(rinv[:], pst[:, 0:1])
                rinvs.append(rinv)
            return rinvs

        def pv_epilogue(qb, pT, rinvs):
            for qt in range(TPC):
                qi = qb * TPC + qt
                pso = psum_att.tile([P, D], F32, tag="o", name=f"o{qb}{qt}")
                for kp in range(NS // 2):
                    nc.tensor.matmul(
                        pso[:],
                        lhsT=pT[:, 2 * kp : 2 * kp + 2, qt * P : (qt + 1) * P],
                        rhs=v_sb[:, 2 * kp : 2 * kp + 2, :],
                        start=(kp == 0),
                        stop=(kp == NS // 2 - 1),
                        perf_mode=DR,
                    )
                t = stage.tile([P, D], F32, tag="t_l", bufs=3)
                nc.vector.tensor_scalar(
                    out=t[:],
                    in0=pso[:],
                    scalar1=rinvs[qt][:],
                    scalar2=gl,
                    op0=OP.mult,
                    op1=OP.mult,
                )
                lsc = stage.tile([P, D], F32, tag="lsc", bufs=3)
                nc.vector.tensor_scalar_mul(
                    out=lsc[:], in0=l_keep[:, qi, :], scalar1=2.0 - gl
                )
                ol = stage.tile([P, D], F32, tag="out_l", bufs=4)
                nc.vector.tensor_add(ol[:], t[:], lsc[:])
                if qt == 0:
                    nc.gpsimd.dma_start(out=out[qi, :, 0:D], in_=ol[:])
                elif qt == 1:
                    nc.scalar.dma_start(out=out[qi, :, 0:D], in_=ol[:])
                elif qt == 2:
                    nc.sync.dma_start(out=out[qi, :, 0:D], in_=ol[:])
                else:
                    nc.gpsimd.dma_start(out=out[qi, :, 0:D], in_=ol[:])

        def a_epilogue(sc):
            for st4 in range(TPC):
                st = sc * TPC + st4
                oa = stage.tile([P, D], F32, tag="oa", bufs=3)
                if st4 % 2 == 0:
                    nc.scalar.mul(oa[:], a_keep[:, st, :], 1.0 + ga)
                else:
                    nc.vector.tensor_scalar_mul(
                        out=oa[:], in0=a_keep[:, st, :], scalar1=1.0 + ga
                    )
                nc.gpsimd.dma_start(out=out[st, :, D : 2 * D], in_=oa[:])

        # ---- phase 0: l0 + a-stream with qb0 scores inlined ----
        l8s = l_load_cast(0)
        l_transp_proj(0, l8s)
        pT0 = ppool.tile([P, NS, QB], FP8, tag="pT", name="pT0")
        for sc in range(NQB):
            a_chunk(sc)
            scores_group(0, pT0, range(sc * TPC, (sc + 1) * TPC))
        vfs, v8s = v_load_cast(0)
        v_rest(0, vfs, v8s)            # PE filler while qb0 exps drain
        rinvs = rowsum(0, pT0)
        nl8s = l_load_cast(1)
        nvfs, nv8s = v_load_cast(1)
        pv_epilogue(0, pT0, rinvs)
        a_epilogue(0)

        # ---- attention slots ----
        for i in range(1, NQB):
            l_transp_proj(i, nl8s)
            v_rest(i, nvfs, nv8s)
            a_epilogue(i)
            pT = ppool.tile([P, NS, QB], FP8, tag="pT", name=f"pT{i}")
            scores_group(i, pT, range(NS))
            rinvs = rowsum(i, pT)
            if i < NQB - 1:
                nl8s = l_load_cast(i + 1)
                nvfs, nv8s = v_load_cast(i + 1)
            pv_epilogue(i, pT, rinvs)

    ctx.close()


def _execute(inputs, trace=False, **run_kwargs):
    a = np.ascontiguousarray(np.asarray(inputs["a"], dtype=np.float32))
    v = np.ascontiguousarray(np.asarray(inputs["v"], dtype=np.float32))
    l = np.ascontiguousarray(np.asarray(inputs["l"], dtype=np.float32))
    Wq = np.asarray(inputs["Wq"], dtype=np.float32)
    Wk = np.asarray(inputs["Wk"], dtype=np.float32)
    Wv = np.asarray(inputs["Wv"], dtype=np.float32)
    W1 = np.asarray(inputs["W1"], dtype=np.float32)
    W2 = np.asarray(inputs["W2"], dtype=np.float32)
    bq = np.asarray(inputs["bq"], dtype=np.float32)
    bk = np.asarray(inputs["bk"], dtype=np.float32)
    bv = np.asarray(inputs["bv"], dtype=np.float32)
    b1 = np.asarray(inputs["b1"], dtype=np.float32)
    b2 = np.asarray(inputs["b2"], dtype=np.float32)
    alpha_a = float(np.asarray(inputs["alpha_a"]))
    alpha_l = float(np.asarray(inputs["alpha_l"]))

    gl = float(1.0 / (1.0 + math.exp(-alpha_l)))
    ga = float(1.0 / (1.0 + math.exp(-alpha_a)))
    b2val = float(b2.reshape(-1)[0])

    nc = build_kernel(gl, ga, b2val)

    FP8NP = ml_dtypes.float8_e4m3
    w2_prep = np.zeros((P, 2, 16), dtype=FP8NP)
    w2_prep[:, :, 0] = W2.reshape(NH, P).T.astype(FP8NP)
    shared = {
        "wq": np.ascontiguousarray(Wq.reshape(NC, P, D).astype(FP8NP)),
        "wk": np.ascontiguousarray(Wk.reshape(NC, P, D).astype(FP8NP)),
        "wv": np.ascontiguousarray(Wv.reshape(NC, P, D).astype(FP8NP)),
        "w1": np.ascontiguousarray(W1.reshape(NC, P, HID).astype(FP8NP)),
        "w2": w2_prep,
        "bq": np.ascontiguousarray(bq.reshape(NC, P).T),
        "bk": np.ascontiguousarray(bk.reshape(NC, P).T),
        "bv": np.ascontiguousarray(bv.reshape(1, D)),
        "b1": np.ascontiguousarray(b1.reshape(NH, P).T),
        "ident8_in": np.eye(P, dtype=FP8NP),
    }
    in_maps = []
    for i in range(B):
        m = dict(shared)
        m["a_t"] = np.ascontiguousarray(a[i].reshape(NS, P, D))
        m["l_t"] = np.ascontiguousarray(l[i].reshape(NS, P, D))
        m["v_t"] = np.ascontiguousarray(v[i].reshape(NS, P, D))
        in_maps.append(m)

    res = run_bass_kernel_spmd(
        nc, in_maps, core_ids=list(range(B)), trace=trace, **run_kwargs
    )
    outs = [res.results[i]["out"].reshape(S, 3 * D) for i in range(B)]
    return np.stack(outs, axis=0).astype(np.float32), res


def kernel(**inputs) -> np.ndarray:
    out, _ = _execute(inputs, trace=False)
    return out


if __name__ == "__main__":
    print("kernel module OK")
